# revision 10
# baseline (speedup 1.0000x reference)
"""Trainium2 Bass kernel for AgnosticNonlinearInteractionBlock (GNN message passing).

Sharding: edges partitioned by receiver node range across 8 cores; each core
computes the full output rows for its 1250-node slice. No collectives.

Per-core device pipeline (bf16 compute, fp32 PSUM accumulation):
  1. linear_up per node  -> up-feature table [N_PAD, 512] bf16 in HBM
  2. per 128-node block: dma_gather sender rows (edges-on-partition layout)
  3. radial MLP (layers 1-3 feature-major, layer 4 via data-as-weights -> edge-major)
  4. CG tensor product via fused scalar_tensor_tensor ops
  5. segment-sum: one-hot scatter matmul into PSUM accumulator
  6. PE-transpose of accumulator -> mid linear -> skip_tp (node-parallel)
Host does only marshalling: edge bucketing/padding, one-hot S, transposes, casts.
"""

import sys

sys.path.insert(0, "/opt/trn_rl_repo")

import numpy as np
import ml_dtypes

BF16 = ml_dtypes.bfloat16

# Problem constants (hardcoded per contest contract)
N, E = 10000, 160000
C, A, R, H = 128, 10, 8, 64
AVG_NEI = 16.0
INV_SQRT3 = 1.0 / np.sqrt(3.0)

NCORES = 8
NPC = N // NCORES          # nodes per core = 1250
BLOCKS = 10                # 128-node blocks per core (1280 padded)
LNPAD = BLOCKS * 128       # 1280 local padded nodes
T_BLK = 18                 # edge tiles (x128) per block, padded
E_BLK = T_BLK * 128        # 2304 edges per block
E_CAP = BLOCKS * E_BLK     # 23040 edges per core
TILES = BLOCKS * T_BLK     # 180 tiles per core
N_PAD = 10112              # 79 * 128 padded node count for up-table
NT_UP = N_PAD // 128       # 79 node tiles for linear_up
MLP_CH = 384               # MLP chunk size (edges) -> psum [64,384] fits 1 bank
N_MSG = 1280               # message columns per edge (m_s2 kept as 3 parts)


def _prep_host(node_attrs, node_feats, edge_attrs, edge_feats, edge_index,
               W_up0, W_up1, W_mlp1, W_mlp2, W_mlp3, W_mlp4,
               W_lin0, W_lin1, W_skip0, W_skip1):
    """Build per-core input arrays (marshalling only, no NN math)."""
    send = np.asarray(edge_index[0]).astype(np.int64)
    recv = np.asarray(edge_index[1]).astype(np.int64)
    ef = np.asarray(edge_feats, dtype=np.float32)
    ea = np.asarray(edge_attrs, dtype=np.float32)

    # ---- shared (replicated) weight arrays, scales folded in ----
    w1 = (np.asarray(W_mlp1, np.float32) / np.sqrt(R)).astype(BF16)          # [8,64]
    w2 = (np.asarray(W_mlp2, np.float32) / np.sqrt(H)).astype(BF16)          # [64,64]
    w3 = (np.asarray(W_mlp3, np.float32) / np.sqrt(H)).astype(BF16)          # [64,64]
    w4 = np.asarray(W_mlp4, np.float32) / np.sqrt(H)                          # [64,512]
    w4 = w4.copy()
    w4[:, C:2 * C] *= INV_SQRT3        # fold w3j coefficient into ws2 block
    w4 = w4.astype(BF16)
    wup = np.stack([np.asarray(W_up0, np.float32), np.asarray(W_up1, np.float32)]
                   ).astype(np.float32) / np.sqrt(C)
    wup = wup.astype(BF16)                                                    # [2,128,128]
    norm = np.sqrt(2 * C) * AVG_NEI
    wl0 = (np.asarray(W_lin0, np.float32) / norm).astype(BF16)                # [256,128]
    wl1 = (np.asarray(W_lin1, np.float32) / norm).astype(BF16)
    # wlin chunks: [4,128,128] = [l0_c0, l0_c1, l1_c0, l1_c1]
    wlin = np.stack([wl0[:C], wl0[C:], wl1[:C], wl1[C:]]).astype(BF16)
    fan = np.sqrt(C * A)
    wsk0 = np.asarray(W_skip0, np.float32) / fan                              # [C,A,C]
    wsk1 = np.asarray(W_skip1, np.float32) / fan
    # wsk chunks: [2,10,128,128]  (x in {s, v}, a)
    wsk = np.stack([wsk0.transpose(1, 0, 2), wsk1.transpose(1, 0, 2)]).astype(BF16)

    # node_feats transposed planes [4,128,N_PAD]: s, v0, v1, v2 component-major
    nfT = np.zeros((4, 128, N_PAD), np.float32)
    nfT[0, :, :N] = np.asarray(node_feats, np.float32)[:, :C].T
    v = np.asarray(node_feats, np.float32)[:, C:].reshape(N, C, 3)
    for i in range(3):
        nfT[1 + i, :, :N] = v[:, :, i].T
    nfT = nfT.reshape(512, N_PAD).astype(BF16)

    ident = np.eye(128, dtype=BF16)

    attrs = np.asarray(node_attrs, np.float32)

    in_maps = []
    for m in range(NCORES):
        lo, hi = m * NPC, (m + 1) * NPC
        mask = (recv >= lo) & (recv < hi)
        eidx = np.nonzero(mask)[0]
        rl = recv[eidx] - lo                      # local node id [0,1250)
        blk = rl // 128
        # bucket edges by block, pad each block to E_BLK
        perm = np.full(E_CAP, -1, np.int64)       # -1 = dummy edge
        rloc = np.zeros(E_CAP, np.int64)          # local recv (within core)
        for b in range(BLOCKS):
            be = eidx[blk == b]
            assert len(be) <= E_BLK, f"core {m} block {b}: {len(be)} > {E_BLK}"
            perm[b * E_BLK: b * E_BLK + len(be)] = be
            rloc[b * E_BLK: b * E_BLK + len(be)] = recv[be] - lo
            rloc[b * E_BLK + len(be): (b + 1) * E_BLK] = b * 128
        real = perm >= 0
        psafe = np.where(real, perm, 0)

        # edge feats transposed [8, E_CAP], zeros for dummies
        efT = np.where(real[None, :], ef[psafe].T, 0.0).astype(BF16)
        # per-edge scalars [y0,y10,y11,y12] tiled: [128, TILES*4]
        ya = np.where(real[:, None], ea[psafe], 0.0).astype(np.float32)
        yq = ya.reshape(TILES, 128, 4).transpose(1, 0, 2).reshape(128, TILES * 4)
        yq = yq.astype(BF16)
        # sender indices wrapped into 16 partitions, replicated to 128
        snd = np.where(real, send[psafe], 0).astype(np.int16)
        sidx = np.zeros((128, TILES * 8), np.int16)  # per block: [128, E_BLK//16]
        for b in range(BLOCKS):
            s_b = snd[b * E_BLK: (b + 1) * E_BLK]
            w = s_b.reshape(E_BLK // 16, 16).T                 # [16, 144]
            sidx[:, b * (E_BLK // 16): (b + 1) * (E_BLK // 16)] = np.tile(w, (8, 1))
        # one-hot scatter matrix [E_CAP, 128] (column = recv mod 128)
        S = np.zeros((E_CAP, 128), np.float32)
        S[np.arange(E_CAP), rloc % 128] = 1.0
        S[~real] = 0.0
        S = S.astype(BF16)
        # replicated node attrs [128, 10*1280]
        arep = np.zeros((A, 128, LNPAD), np.float32)
        arep[:, :, :NPC] = attrs[lo:hi].T[:, None, :]
        arep = arep.reshape(A, 128, LNPAD).transpose(1, 0, 2).reshape(128, A * LNPAD)
        arep = arep.astype(BF16)

        in_maps.append(dict(
            efT=efT, yq=yq, sidx=sidx, S=S, arep=arep, nfT=nfT,
            w1=w1, w2=w2, w3=w3, w4=w4,
            wup=wup.reshape(256, 128), wlin=wlin.reshape(512, 128),
            wsk=wsk.reshape(2560, 128), ident=ident,
        ))
    return in_maps


def _assemble_output(results):
    """results: per-core dict with 'out' [512, 1280] f32 -> full [N, 512] f32."""
    out = np.zeros((N, 4 * C), np.float32)
    for m in range(NCORES):
        o = np.asarray(results[m]["out"], np.float32)[:, :NPC]   # [512, 1250]
        lo = m * NPC
        out[lo:lo + NPC, :C] = o[:C].T                            # o_s
        for i in range(3):
            out[lo:lo + NPC, C + i::3] = o[C * (1 + i):C * (2 + i)].T
    return out


# ---------------------------------------------------------------------------
# Device kernel builder
# ---------------------------------------------------------------------------

_CACHE = {}


def _build_nc():
    from concourse import bass, bacc, tile, mybir

    dt = mybir.dt
    AF = mybir.ActivationFunctionType
    OP = mybir.AluOpType

    nc = bacc.Bacc("TRN2", target_bir_lowering=False, debug=False,
                   num_devices=NCORES)

    # DRAM I/O
    d_efT = nc.dram_tensor("efT", [8, E_CAP], dt.bfloat16, kind="ExternalInput")
    d_yq = nc.dram_tensor("yq", [128, TILES * 4], dt.bfloat16, kind="ExternalInput")
    d_sidx = nc.dram_tensor("sidx", [128, TILES * 8], dt.int16, kind="ExternalInput")
    d_S = nc.dram_tensor("S", [E_CAP, 128], dt.bfloat16, kind="ExternalInput")
    d_arep = nc.dram_tensor("arep", [128, A * LNPAD], dt.bfloat16, kind="ExternalInput")
    d_nfT = nc.dram_tensor("nfT", [512, N_PAD], dt.bfloat16, kind="ExternalInput")
    d_w1 = nc.dram_tensor("w1", [8, 64], dt.bfloat16, kind="ExternalInput")
    d_w2 = nc.dram_tensor("w2", [64, 64], dt.bfloat16, kind="ExternalInput")
    d_w3 = nc.dram_tensor("w3", [64, 64], dt.bfloat16, kind="ExternalInput")
    d_w4 = nc.dram_tensor("w4", [64, 512], dt.bfloat16, kind="ExternalInput")
    d_wup = nc.dram_tensor("wup", [256, 128], dt.bfloat16, kind="ExternalInput")
    d_wlin = nc.dram_tensor("wlin", [512, 128], dt.bfloat16, kind="ExternalInput")
    d_wsk = nc.dram_tensor("wsk", [2560, 128], dt.bfloat16, kind="ExternalInput")
    d_ident = nc.dram_tensor("ident", [128, 128], dt.bfloat16, kind="ExternalInput")
    d_out = nc.dram_tensor("out", [512, LNPAD], dt.float32, kind="ExternalOutput")
    d_table = nc.dram_tensor("table", [N_PAD, 512], dt.bfloat16, kind="Internal")

    with tile.TileContext(nc) as tc:
        with (
            tc.tile_pool(name="const", bufs=1) as cpool,
            tc.tile_pool(name="work", bufs=3) as wpool,
            tc.tile_pool(name="gbuf", bufs=2) as gpool,
            tc.tile_pool(name="msg", bufs=3) as mpool,
            tc.tile_pool(name="blk", bufs=2) as bpool,
            tc.tile_pool(name="psA", bufs=4, space=bass.MemorySpace.PSUM) as psA,
            tc.tile_pool(name="psB", bufs=1, space=bass.MemorySpace.PSUM) as psB,
        ):
            # ---- resident constants ----
            efT = cpool.tile([8, E_CAP], dt.bfloat16)
            nc.sync.dma_start(efT[:], d_efT[:])
            yq = cpool.tile([128, TILES * 4], dt.bfloat16)
            nc.sync.dma_start(yq[:], d_yq[:])
            sidx = cpool.tile([128, TILES * 8], dt.int16)
            nc.sync.dma_start(sidx[:], d_sidx[:])
            arep = cpool.tile([128, A * LNPAD], dt.bfloat16)
            nc.sync.dma_start(arep[:], d_arep[:])
            w1 = cpool.tile([8, 64], dt.bfloat16)
            nc.sync.dma_start(w1[:], d_w1[:])
            w2 = cpool.tile([64, 64], dt.bfloat16)
            nc.sync.dma_start(w2[:], d_w2[:])
            w3 = cpool.tile([64, 64], dt.bfloat16)
            nc.sync.dma_start(w3[:], d_w3[:])
            w4 = cpool.tile([64, 512], dt.bfloat16)
            nc.sync.dma_start(w4[:], d_w4[:])
            wup = cpool.tile([128, 256], dt.bfloat16)   # [128, 2*128]
            for k in range(2):
                nc.sync.dma_start(wup[:, k * 128:(k + 1) * 128],
                                  d_wup[k * 128:(k + 1) * 128, :])
            wlin = cpool.tile([128, 512], dt.bfloat16)  # [128, 4*128]
            for k in range(4):
                nc.sync.dma_start(wlin[:, k * 128:(k + 1) * 128],
                                  d_wlin[k * 128:(k + 1) * 128, :])
            wsk = cpool.tile([128, 2560], dt.bfloat16)  # [128, 20*128]
            for k in range(20):
                nc.sync.dma_start(wsk[:, k * 128:(k + 1) * 128],
                                  d_wsk[k * 128:(k + 1) * 128, :])
            ident = cpool.tile([128, 128], dt.bfloat16)
            nc.sync.dma_start(ident[:], d_ident[:])

            # ---- Phase 1: linear_up -> table[N_PAD, 512] ----
            for g in range(NT_UP // 4 + (1 if NT_UP % 4 else 0)):
                nts = range(g * 4, min((g + 1) * 4, NT_UP))
                slabs = []
                for comp in range(4):
                    slab = wpool.tile([128, 512], dt.bfloat16, tag="upslab")
                    w = len(nts) * 128
                    nc.sync.dma_start(
                        slab[:, :w],
                        d_nfT[comp * 128:(comp + 1) * 128,
                              nts[0] * 128: nts[0] * 128 + w])
                    slabs.append(slab)
                for j, nt in enumerate(nts):
                    ps = psA.tile([128, 512], dt.float32, tag="psA")
                    for comp in range(4):
                        nc.tensor.matmul(
                            ps[:, comp * 128:(comp + 1) * 128],
                            slabs[comp][:, j * 128:(j + 1) * 128],
                            wup[:, (0 if comp == 0 else 128):(128 if comp == 0 else 256)],
                            start=True, stop=True)
                    up_sb = wpool.tile([128, 512], dt.bfloat16, tag="upsb")
                    nc.scalar.activation(up_sb[:], ps[:], AF.Copy)
                    nc.sync.dma_start(d_table[nt * 128:(nt + 1) * 128, :], up_sb[:])

            # ---- Phase 2: per-block message passing ----
            osb = cpool.tile([128, BLOCKS * 512], dt.bfloat16)  # mid-linear out planes
            for b in range(BLOCKS):
                gb = gpool.tile([128, T_BLK * 512], dt.bfloat16, tag="gather")
                import os as _os
                if _os.environ.get("ABLATE") == "nogather":
                    # wrong data, same shapes: sequential rows instead of gather
                    nc.sync.dma_start(
                        gb[:].rearrange("p (t e) -> p t e", t=T_BLK),
                        d_table[:, :].rearrange("(t p) e -> p t e", p=128)[:, 0:T_BLK, :])
                else:
                    # chunked gathers: keep SWDGE descriptor count per call
                    # well under the dynamic-DMA scratch ring capacity
                    GCH = 768          # edges per gather (6 tiles), mult of 128
                    for gi in range(E_BLK // GCH):
                        nc.gpsimd.dma_gather(
                            out_ap=gb[:, gi * GCH * 512 // 128:
                                    (gi + 1) * GCH * 512 // 128].rearrange(
                                        "p (t e) -> p t e", e=512),
                            in_ap=d_table[:, :],
                            idxs_ap=sidx[:, (b * E_BLK + gi * GCH) // 16:
                                         (b * E_BLK + (gi + 1) * GCH) // 16],
                            num_idxs=GCH, num_idxs_reg=GCH, elem_size=512,
                        )

                macc = psB.tile([128, N_MSG], dt.float32, tag="psB")

                for ch in range(E_BLK // MLP_CH):       # 6 chunks of 384 edges
                    e0 = b * E_BLK + ch * MLP_CH
                    h = psA.tile([64, MLP_CH], dt.float32, tag="psA")
                    nc.tensor.matmul(h[:], w1[:], efT[:, e0:e0 + MLP_CH],
                                     start=True, stop=True)
                    h1 = wpool.tile([64, MLP_CH], dt.bfloat16, tag="h1")
                    nc.scalar.activation(h1[:], h[:], AF.Silu)
                    h = psA.tile([64, MLP_CH], dt.float32, tag="psA")
                    nc.tensor.matmul(h[:], w2[:], h1[:], start=True, stop=True)
                    h2 = wpool.tile([64, MLP_CH], dt.bfloat16, tag="h2")
                    nc.scalar.activation(h2[:], h[:], AF.Silu)
                    h = psA.tile([64, MLP_CH], dt.float32, tag="psA")
                    nc.tensor.matmul(h[:], w3[:], h2[:], start=True, stop=True)
                    h3 = wpool.tile([64, MLP_CH], dt.bfloat16, tag="h3")
                    nc.scalar.activation(h3[:], h[:], AF.Silu)

                    for s in range(MLP_CH // 128):      # 3 edge tiles per chunk
                        t_loc = ch * 3 + s
                        t = b * T_BLK + t_loc
                        wt_ps = psA.tile([128, 512], dt.float32, tag="psA")
                        nc.tensor.matmul(wt_ps[:], h3[:, s * 128:(s + 1) * 128],
                                         w4[:], start=True, stop=True)
                        wt = wpool.tile([128, 512], dt.bfloat16, tag="wt")
                        nc.scalar.activation(wt[:], wt_ps[:], AF.Copy)

                        G = gb[:, t_loc * 512:(t_loc + 1) * 512]
                        msg = mpool.tile([128, N_MSG], dt.bfloat16, tag="msg")
                        y0 = yq[:, t * 4: t * 4 + 1]
                        # m_s1 = (s*y0)*ws1
                        nc.vector.scalar_tensor_tensor(
                            msg[:, 0:128], G[:, 0:128], y0, wt[:, 0:128],
                            OP.mult, OP.mult)
                        # m_v2_i = (v_i*y0)*wv2
                        nc.vector.scalar_tensor_tensor(
                            msg[:, 128:512].rearrange("p (i c) -> p i c", i=3),
                            G[:, 128:512].rearrange("p (i c) -> p i c", i=3),
                            y0,
                            wt[:, 384:512].unsqueeze(1).broadcast_to((128, 3, 128)),
                            OP.mult, OP.mult)
                        for i in range(3):
                            y1i = yq[:, t * 4 + 1 + i: t * 4 + 2 + i]
                            # m_v1_i = (s*y1i)*wv1
                            nc.vector.scalar_tensor_tensor(
                                msg[:, 512 + i * 128: 640 + i * 128],
                                G[:, 0:128], y1i, wt[:, 256:384],
                                OP.mult, OP.mult)
                            # m_s2_i = (v_i*y1i)*ws2
                            nc.vector.scalar_tensor_tensor(
                                msg[:, 896 + i * 128: 1024 + i * 128],
                                G[:, 128 + i * 128: 256 + i * 128], y1i,
                                wt[:, 128:256], OP.mult, OP.mult)

                        Ssb = mpool.tile([128, 128], dt.bfloat16, tag="S")
                        nc.sync.dma_start(Ssb[:], d_S[t * 128:(t + 1) * 128, :])
                        # PSUM-bank-aligned N<=512 sub-matmuls
                        for c0, c1 in ((0, 512), (512, 1024), (1024, 1280)):
                            nc.tensor.matmul(macc[:, c0:c1], Ssb[:], msg[:, c0:c1],
                                             start=(t_loc == 0),
                                             stop=(t_loc == T_BLK - 1),
                                             skip_group_check=True)

                # ---- per-block: transpose Macc, mid linear ----
                msb = bpool.tile([128, N_MSG], dt.bfloat16, tag="msb")
                nc.vector.tensor_copy(msb[:], macc[:])
                MT = bpool.tile([128, 10 * 128], dt.bfloat16, tag="MT")
                for k in range(10):
                    tp = psA.tile([128, 128], dt.bfloat16, tag="psA")
                    nc.tensor.transpose(tp[:], msb[:, k * 128:(k + 1) * 128],
                                        ident[:])
                    nc.scalar.activation(MT[:, k * 128:(k + 1) * 128], tp[:],
                                         AF.Copy)
                pm = psA.tile([128, 512], dt.float32, tag="psA")
                # out_sT = wl0c0.T@MT[ms1] + wl0c1.T@(MT[7]+MT[8]+MT[9])
                nc.tensor.matmul(pm[:, 0:128], wlin[:, 0:128], MT[:, 0:128],
                                 start=True, stop=False, skip_group_check=True)
                for k in (7, 8, 9):
                    nc.tensor.matmul(pm[:, 0:128], wlin[:, 128:256],
                                     MT[:, k * 128:(k + 1) * 128],
                                     start=False, stop=(k == 9),
                                     skip_group_check=True)
                # out_v_iT = wl1c0.T@MT[4+i] + wl1c1.T@MT[1+i]
                for i in range(3):
                    nc.tensor.matmul(pm[:, 128 * (1 + i):128 * (2 + i)],
                                     wlin[:, 256:384],
                                     MT[:, (4 + i) * 128:(5 + i) * 128],
                                     start=True, stop=False, skip_group_check=True)
                    nc.tensor.matmul(pm[:, 128 * (1 + i):128 * (2 + i)],
                                     wlin[:, 384:512],
                                     MT[:, (1 + i) * 128:(2 + i) * 128],
                                     start=False, stop=True, skip_group_check=True)
                nc.scalar.activation(osb[:, b * 512:(b + 1) * 512], pm[:], AF.Copy)

            # ---- Phase 3: skip_tp (N<=512/bank: 5 groups of 2 blocks) ----
            HNODES = LNPAD // 5
            for hf in range(5):
                for x in range(4):
                    po = psB.tile([128, HNODES], dt.float32, tag="psB")
                    osb_x = osb[:].rearrange(
                        "p (b c) -> p b c", c=512)[:, hf * 2:(hf + 1) * 2,
                                                   x * 128:(x + 1) * 128]
                    for a in range(A):
                        Z = wpool.tile([128, HNODES], dt.bfloat16, tag="Z")
                        ar = arep[:, a * LNPAD + hf * HNODES:
                                  a * LNPAD + (hf + 1) * HNODES]
                        nc.vector.tensor_tensor(
                            Z[:].rearrange("p (b c) -> p b c", c=128),
                            osb_x, ar.rearrange("p (b c) -> p b c", c=128),
                            mybir.AluOpType.mult)
                        wchunk = wsk[:, ((0 if x == 0 else 10) + a) * 128:
                                     ((0 if x == 0 else 10) + a) * 128 + 128]
                        nc.tensor.matmul(po[:], wchunk, Z[:],
                                         start=(a == 0), stop=(a == A - 1),
                                         skip_group_check=True)
                    oout = wpool.tile([128, HNODES], dt.float32, tag="oout")
                    nc.vector.tensor_copy(oout[:], po[:])
                    nc.sync.dma_start(
                        d_out[x * 128:(x + 1) * 128,
                              hf * HNODES:(hf + 1) * HNODES], oout[:])

    nc.compile()
    return nc


def kernel(**inputs):
    in_maps = _prep_host(**inputs)
    if "nc" not in _CACHE:
        _CACHE["nc"] = _build_nc()
    nc = _CACHE["nc"]
    from concourse.bass_utils import run_bass_kernel_spmd
    res = run_bass_kernel_spmd(nc, in_maps, core_ids=list(range(NCORES)))
    return _assemble_output(res.results)
